# revision 1
# baseline (speedup 1.0000x reference)
"""Trainium2 Bass kernel v5: full softmax attention, engine-balanced.

v4 -> v5:
  - last two k-tiles of each sb exp'd on DVE (kills the ACT-backlog stall on
    the next sb's first QKs; psum slots rotate continuously across sbs)
  - sb epilogue order: PV(last), ot drain copy, then tree L0 chunk 3;
    tree tail (L1b/L2/L3 + lr DMA) deferred into the next sb's slack
  - final sb: drain on ACT, final DMAs split in halves across queues
  - startup: first DMA chunks ordered/need-sized so QK(0) starts early
"""

import numpy as np
import ml_dtypes
from contextlib import ExitStack

import concourse.bass as bass
import concourse.bacc as bacc
import concourse.mybir as mybir
import concourse.tile as tile
from concourse.bass_utils import run_bass_kernel_spmd

B, S, H, D = 1, 4096, 16, 128
N_CORES = 8
HPC = H // N_CORES
SB = 1024
NSB = S // SB
NKT = S // 128
SCALE = float(1.0 / np.sqrt(D))
BF16 = mybir.dt.bfloat16
FP32 = mybir.dt.float32
I16 = mybir.dt.int16

SCH_SIGMA = 0.05754
SCH_A = float(SCALE * 128.0 / np.log(2.0))
SCH_B = float(128.0 * (127.0 - SCH_SIGMA))
DVE_SET = frozenset((3, 7, 11, 15, 19, 23, 27, 31))

_CACHE = {}


def _build():
    nc = bacc.Bacc("TRN2", target_bir_lowering=False, debug=False)
    qt_d = nc.dram_tensor("qt", [HPC, 128, S], BF16, kind="ExternalInput")
    kt_d = nc.dram_tensor("kt", [HPC, 128, S], BF16, kind="ExternalInput")
    vp_d = nc.dram_tensor("vp", [HPC, 128, S], BF16, kind="ExternalInput")
    o_d = nc.dram_tensor("o", [HPC, NSB, 128, SB], FP32, kind="ExternalOutput")
    lr_d = nc.dram_tensor("lr", [HPC, NSB, 128, 2 * SB], BF16, kind="ExternalOutput")

    with ExitStack() as ctx:
        tc = ctx.enter_context(tile.TileContext(nc))
        qkv = ctx.enter_context(tc.tile_pool(name="qkv", bufs=2))
        ptp = ctx.enter_context(tc.tile_pool(name="ptp", bufs=1))
        trp = ctx.enter_context(tc.tile_pool(name="trp", bufs=1))
        drp = ctx.enter_context(tc.tile_pool(name="drp", bufs=2))

        scp = ctx.enter_context(tc.tile_pool(name="scp", bufs=3, space="PSUM"))
        otp = ctx.enter_context(tc.tile_pool(name="otp", bufs=1, space="PSUM"))

        wsrc = qkv.tile([128, 512], BF16, name="wsrc", tag="wsrc")
        nc.vector.memset(wsrc, 1.0)
        wsc = scp.tile([128, SB], FP32, name="wsc", tag="sc")
        for wi in range(14):
            nc.tensor.matmul(wsc[:, (wi % 2) * 512:(wi % 2) * 512 + 512],
                             wsrc[:, :128], wsrc, start=True, stop=True)

        deferred = []
        pvq = []
        for h in range(HPC):
            qt_s = qkv.tile([128, S], BF16, name=f"qt{h}", tag="qt")
            kt_s = qkv.tile([128, S], BF16, name=f"kt{h}", tag="kt")
            v_s = qkv.tile([128, S], BF16, name=f"v{h}", tag="v")
            if h == 0:
                # need-ordered startup: kt k-tile0 + first q half, then rest
                nc.sync.dma_start(kt_s[:, 0:128], kt_d[h][:, 0:128])
                nc.sync.dma_start(qt_s[:, 0:512], qt_d[h][:, 0:512])
                nc.sync.dma_start(kt_s[:, 128:512], kt_d[h][:, 128:512])
                nc.sync.dma_start(qt_s[:, 512:1024], qt_d[h][:, 512:1024])
                nc.sync.dma_start(v_s[:, 0:512], vp_d[h][:, 0:512])
                bounds = [512, 1024, 2048, 3072, 4096]
                for a, b in zip(bounds[:-1], bounds[1:]):
                    nc.sync.dma_start(kt_s[:, a:b], kt_d[h][:, a:b])
                    if a >= 1024:
                        nc.sync.dma_start(qt_s[:, a:b], qt_d[h][:, a:b])
                    nc.sync.dma_start(v_s[:, a:b], vp_d[h][:, a:b])
            else:
                for a, b in [(0, 1024), (1024, 2048), (2048, 3072), (3072, 4096)]:
                    nc.sync.dma_start(kt_s[:, a:b], kt_d[h][:, a:b])
                    nc.sync.dma_start(qt_s[:, a:b], qt_d[h][:, a:b])
                    nc.sync.dma_start(v_s[:, a:b], vp_d[h][:, a:b])

            for sb in range(NSB):
                q0 = sb * SB
                last = (h == HPC - 1) and (sb == NSB - 1)
                ot = otp.tile([128, SB], FP32, name=f"ot_{h}_{sb}", tag="ot")
                pt = ptp.tile([128, NKT * SB], BF16, name=f"pt_{h}_{sb}", tag="pt")
                pt_i16 = pt.bitcast(I16)
                t1 = trp.tile([128, 16 * SB], BF16, name=f"t1_{h}_{sb}", tag="t1")
                t2 = trp.tile([128, 8 * SB], BF16, name=f"t2_{h}_{sb}", tag="t2")
                t3 = trp.tile([128, 4 * SB], BF16, name=f"t3_{h}_{sb}", tag="t3")
                t4 = trp.tile([128, 2 * SB], BF16, name=f"t4_{h}_{sb}", tag="t4")

                def pv(j, ot=ot, pt=pt, v_s=v_s):
                    vj = v_s[:, j * 128:(j + 1) * 128]
                    pj = pt[:, j * SB:(j + 1) * SB]
                    nc.tensor.matmul(ot[:, :512], vj, pj[:, :512],
                                     start=(j == 0), stop=(j == NKT - 1))
                    nc.tensor.matmul(ot[:, 512:], vj, pj[:, 512:],
                                     start=(j == 0), stop=(j == NKT - 1))

                def l0chunk(c, pt=pt, t1=t1):
                    for hh2 in range(2):
                        o2 = (2 * c + hh2) * 4 * SB
                        src = pt[:, o2:o2 + 4 * SB].rearrange(
                            "p (t two q) -> p t two q", two=2, q=SB)
                        dst = t1[:, o2 // 2:o2 // 2 + 2 * SB].rearrange(
                            "p (t q) -> p t q", q=SB)
                        nc.vector.tensor_add(dst, src[:, :, 0, :], src[:, :, 1, :])

                for j in range(NKT):
                    sc = scp.tile([128, SB], FP32, name=f"sc_{h}_{sb}_{j}", tag="sc")
                    kj = kt_s[:, j * 128:(j + 1) * 128]
                    nc.tensor.matmul(sc[:, :512], kj, qt_s[:, q0:q0 + 512],
                                     start=True, stop=True)
                    nc.tensor.matmul(sc[:, 512:], kj, qt_s[:, q0 + 512:q0 + SB],
                                     start=True, stop=True)
                    if j in DVE_SET:
                        nc.vector.tensor_scalar(
                            pt_i16[:, j * SB:(j + 1) * SB], sc, SCH_A, SCH_B,
                            mybir.AluOpType.mult, mybir.AluOpType.add)
                    else:
                        nc.scalar.activation(
                            pt[:, j * SB:(j + 1) * SB], sc,
                            mybir.ActivationFunctionType.Exp, scale=SCALE)
                    if j % 8 == 7 and j < 31:
                        l0chunk(j // 8)
                    if j == 15:
                        for hh2 in range(2):
                            o2 = hh2 * 4 * SB
                            src = t1[:, o2:o2 + 4 * SB].rearrange(
                                "p (t two q) -> p t two q", two=2, q=SB)
                            dst = t2[:, o2 // 2:o2 // 2 + 2 * SB].rearrange(
                                "p (t q) -> p t q", q=SB)
                            nc.vector.tensor_add(dst, src[:, :, 0, :],
                                                 src[:, :, 1, :])
                    if deferred and j in (4, 12, 20):
                        deferred.pop(0)()

                    def pvstep(j=j, pv=pv, h=h, sb=sb, ot=ot, last=last,
                               l0chunk=l0chunk):
                        pv(j)
                        if j == NKT - 1:
                            # sb epilogue rides with the last PV: drain + L0c3
                            osb = drp.tile([128, SB], FP32,
                                           name=f"osb_{h}_{sb}", tag="osb")
                            for qq in range(4):
                                cs = slice(qq * SB // 4, (qq + 1) * SB // 4)
                                if last:
                                    nc.scalar.copy(osb[:, cs], ot[:, cs])
                                else:
                                    nc.vector.tensor_copy(osb[:, cs], ot[:, cs])
                                nc.sync.dma_start(o_d[h, sb][:, cs], osb[:, cs])
                            l0chunk(3)
                    pvq.append(pvstep)
                    if len(pvq) > 3:
                        pvq.pop(0)()

                def tail1(t1=t1, t2=t2):
                    for hh2 in range(2):
                        o2 = (2 + hh2) * 4 * SB
                        src = t1[:, o2:o2 + 4 * SB].rearrange(
                            "p (t two q) -> p t two q", two=2, q=SB)
                        dst = t2[:, o2 // 2:o2 // 2 + 2 * SB].rearrange(
                            "p (t q) -> p t q", q=SB)
                        nc.vector.tensor_add(dst, src[:, :, 0, :], src[:, :, 1, :])
                def tail2(t2=t2, t3=t3):
                    for hh2 in range(2):
                        o2 = hh2 * 4 * SB
                        src2 = t2[:, o2:o2 + 4 * SB].rearrange(
                            "p (t two q) -> p t two q", two=2, q=SB)
                        t3v = t3[:, o2 // 2:o2 // 2 + 2 * SB].rearrange(
                            "p (t q) -> p t q", q=SB)
                        nc.vector.tensor_add(t3v, src2[:, :, 0, :], src2[:, :, 1, :])
                def tail3(h=h, sb=sb, t3=t3, t4=t4, last=last):
                    src3 = t3.rearrange("p (t two q) -> p t two q", two=2, q=SB)
                    t4v = t4.rearrange("p (t q) -> p t q", q=SB)
                    nc.vector.tensor_add(t4v, src3[:, :, 0, :], src3[:, :, 1, :])
                    if last:
                        for qq in range(4):
                            cs = slice(qq * SB // 2, (qq + 1) * SB // 2)
                            nc.sync.dma_start(lr_d[h, sb][:, cs], t4[:, cs])
                    else:
                        nc.sync.dma_start(lr_d[h, sb], t4)
                if last:
                    while pvq:
                        pvq.pop(0)()
                    tail1(); tail2(); tail3()
                else:
                    deferred.extend([tail1, tail2, tail3])
        while pvq:
            pvq.pop(0)()
        while deferred:
            deferred.pop(0)()
    nc.compile()
    return nc


def _prep_inputs(q, k, v):
    bf = ml_dtypes.bfloat16
    in_maps = []
    for c in range(N_CORES):
        hs = slice(c * HPC, (c + 1) * HPC)
        qt = np.transpose(q[:, hs, :], (1, 2, 0)).astype(bf)
        kt = np.transpose(k[:, hs, :], (1, 2, 0)).astype(bf)
        vh = np.transpose(v[:, hs, :], (1, 0, 2))
        vp = np.ascontiguousarray(
            vh.reshape(HPC, S // 128, 128, D).transpose(0, 2, 1, 3)
        ).reshape(HPC, 128, S).astype(bf)
        in_maps.append({"qt": qt, "kt": kt, "vp": vp})
    return in_maps


def kernel(q, k, v, ring_size=None, **_unused):
    q = np.asarray(q, dtype=np.float32).reshape(S, H, D)
    k = np.asarray(k, dtype=np.float32).reshape(S, H, D)
    v = np.asarray(v, dtype=np.float32).reshape(S, H, D)

    in_maps = _prep_inputs(q, k, v)
    if "nc" not in _CACHE:
        _CACHE["nc"] = _build()
    res = run_bass_kernel_spmd(_CACHE["nc"], in_maps, list(range(N_CORES))).results

    out = np.empty((B, S, H, D), np.float32)
    for c in range(N_CORES):
        o = np.asarray(res[c]["o"])
        lr = np.asarray(res[c]["lr"]).astype(np.float32)
        for hh in range(HPC):
            l = lr[hh].reshape(NSB, 128, 2, SB).sum(axis=(1, 2))
            on = o[hh] / l[:, None, :]
            out[0, :, c * HPC + hh, :] = on.transpose(0, 2, 1).reshape(S, D)
    return out

